# revision 1
# baseline (speedup 1.0000x reference)
"""CircularMaxPool2d (disk stencil, radius 5, reflect padding) on 8 TRN2 NeuronCores.

Input x: [8, 1, 2048, 2048] f32. Data-parallel: core c processes batch c.

Algorithm (exact fp32): decompose the disk mask by rows. For radius 5 the
disk rows are: dy=0 -> 11-wide, |dy| in {1,2,3} -> 9-wide, |dy|=4 -> 7-wide,
|dy|=5 -> 1-wide. So

  out[i,j] = max( h5[i,j], max_{|d|<=3} h4[i+d,j], h3[i-4,j], h3[i+4,j],
                  x[i-5,j], x[i+5,j] )

where hk = horizontal (2k+1)-wide running max of x. Horizontal maxes are
built with a shared doubling ladder (s1=2,s2=4,s3=6-wide); the vertical
combination uses a 2-level ladder for the h4 band plus direct taps. All ops
are free-dim DVE tensor_tensor maxes (fp32 tensor_tensor = 1 elem/cyc/lane;
this kernel is DVE-bound, DMA fully hidden).

Layout: each partition owns a (column-chunk, row-group) pair: G=32
consecutive rows x WB=128 columns. The input is packed on the host into a
blocked [superband, 128, G+10, WB+10] tensor with vertical halo rows and
reflect padding baked in, so every HBM load is fully contiguous and every
vertical shift is a free-dim offset. The horizontal ladder is computed on
the halo rows too (DVE cannot read partition-shifted operands, and
partition-shifted SBUF->SBUF DMA is slow ~22GB/s), so the kernel needs no
on-device halo exchange at all. Output is written blocked and unscrambled
on the host.
"""

import sys

sys.path.insert(0, "/opt/trn_rl_repo")

import numpy as np

H = 2048
W = 2048
RAD = 5
P = 128
G = 64  # rows per partition group
NG = H // G  # row groups
NCHUNK = P // NG  # column chunks per superband
WB = 64  # cols per chunk
WH = WB + 2 * RAD  # 138
NSB = W // (WB * NCHUNK)  # 8 superbands
XR = G + 2 * RAD  # 42 rows in x tile
N_CORES = 8

_CACHE = {}


def _build():
    import concourse.bacc as bacc
    import concourse.tile as tile
    import concourse.mybir as mybir

    f32 = mybir.dt.float32
    MAX = mybir.AluOpType.max

    nc = bacc.Bacc("TRN2", target_bir_lowering=False, debug=False, num_devices=N_CORES)
    xin = nc.dram_tensor("xin", [NSB, P, XR, WH], f32, kind="ExternalInput").ap()
    yout = nc.dram_tensor("yout", [NSB, P, G, WB], f32, kind="ExternalOutput").ap()

    with tile.TileContext(nc) as tc:
        with (
            tc.tile_pool(name="xx", bufs=2) as p_xx,
            tc.tile_pool(name="ladA", bufs=1) as p_a,
            tc.tile_pool(name="ladB", bufs=1) as p_b,
            tc.tile_pool(name="h4x", bufs=1) as p_h4,
            tc.tile_pool(name="h3x", bufs=1) as p_h3,
            tc.tile_pool(name="acc", bufs=2) as p_acc,
        ):
            for b in range(NSB):
                # ---- load packed x band (rows Gp-5..Gp+G+4, halos pre-baked)
                xx = p_xx.tile([P, XR, WH], f32, tag="xx")
                s1 = p_a.tile([P, G + 8, WH], f32, tag="A")
                if b == 0:
                    # split the cold-start load so the ladder starts after the
                    # first half lands (trims the pipeline ramp)
                    hs = XR // 2  # 37
                    nc.sync.dma_start(xx[:, 0:hs, :], xin[b][:, 0:hs, :])
                    nc.sync.dma_start(xx[:, hs:XR, :], xin[b][:, hs:XR, :])
                    nc.vector.tensor_tensor(
                        s1[:, 0 : hs - 1, 0 : WH - 1],
                        xx[:, 1:hs, 0 : WH - 1],
                        xx[:, 1:hs, 1:WH],
                        op=MAX,
                    )
                    nc.vector.tensor_tensor(
                        s1[:, hs - 1 : G + 8, 0 : WH - 1],
                        xx[:, hs : G + 9, 0 : WH - 1],
                        xx[:, hs : G + 9, 1:WH],
                        op=MAX,
                    )
                else:
                    nc.sync.dma_start(xx[:, :, :], xin[b])
                    # ---- horizontal ladder on rows -4..G+3 (xx slots 1..G+8)
                    # s1/s2 rows -4..G+3 (G+8, slot = r+4); s3 rows -3..G+2 (G+6, slot = r+3)
                    nc.vector.tensor_tensor(
                        s1[:, :, 0 : WH - 1],
                        xx[:, 1 : G + 9, 0 : WH - 1],
                        xx[:, 1 : G + 9, 1:WH],
                        op=MAX,
                    )
                s2 = p_b.tile([P, G + 8, WH], f32, tag="B")
                nc.vector.tensor_tensor(
                    s2[:, :, 0 : WH - 3],
                    s1[:, :, 0 : WH - 3],
                    s1[:, :, 2 : WH - 1],
                    op=MAX,
                )
                s3 = p_a.tile([P, G + 6, WH], f32, tag="A")
                nc.vector.tensor_tensor(
                    s3[:, :, 0 : WH - 5],
                    s2[:, 1 : G + 7, 0 : WH - 5],
                    s2[:, 1 : G + 7, 2 : WH - 3],
                    op=MAX,
                )
                # h3 (7-wide, used at dy=+-4), rows -4..G+3 (slot = r+4)
                h3x = p_h3.tile([P, G + 8, WB], f32, tag="h3x")
                nc.vector.tensor_tensor(
                    h3x[:, :, :], s2[:, :, 2 : 2 + WB], s2[:, :, 5 : 5 + WB], op=MAX
                )
                # h5 (11-wide, dy=0) straight into the accumulator (rows 0..G-1)
                acc = p_acc.tile([P, G, WB], f32, tag="acc")
                nc.vector.tensor_tensor(
                    acc[:, :, :],
                    s3[:, 3 : G + 3, 0:WB],
                    s3[:, 3 : G + 3, 5 : 5 + WB],
                    op=MAX,
                )
                # h4 (9-wide, |dy|<=3), rows -3..G+2 (slot = r+3)
                h4x = p_h4.tile([P, G + 6, WB], f32, tag="h4x")
                nc.vector.tensor_tensor(
                    h4x[:, :, :], s3[:, :, 1 : 1 + WB], s3[:, :, 4 : 4 + WB], op=MAX
                )

                # ---- vertical combine
                # t1[r] = max(h4[r], h4[r+1]) for r in -3..G+1  (slot = r+3)
                t1 = p_b.tile([P, G + 5, WB], f32, tag="B")
                nc.vector.tensor_tensor(
                    t1[:, :, :], h4x[:, 0 : G + 5, :], h4x[:, 1 : G + 6, :], op=MAX
                )
                # t2[r] = max(t1[r], t1[r+2]) = max h4[r..r+3], r in -3..G-1 (slot = r+3)
                t2 = p_a.tile([P, G + 3, WB], f32, tag="A")
                nc.vector.tensor_tensor(
                    t2[:, :, :], t1[:, 0 : G + 3, :], t1[:, 2 : G + 5, :], op=MAX
                )
                # acc = max(acc, t2[r-3], t2[r])  -> max over h4[r-3..r+3]
                nc.vector.tensor_tensor(acc[:], acc[:], t2[:, 0:G, :], op=MAX)
                nc.vector.tensor_tensor(acc[:], acc[:], t2[:, 3 : G + 3, :], op=MAX)
                # h3 taps at dy = -4, +4 (slot = r-+4 + 4)
                nc.vector.tensor_tensor(acc[:], acc[:], h3x[:, 0:G, :], op=MAX)
                nc.vector.tensor_tensor(acc[:], acc[:], h3x[:, 8 : G + 8, :], op=MAX)
                # x taps at dy = -5, +5 (xx slot = r-+5 + 5, col offset +5)
                nc.vector.tensor_tensor(
                    acc[:], acc[:], xx[:, 0:G, 5 : 5 + WB], op=MAX
                )
                nc.vector.tensor_tensor(
                    acc[:], acc[:], xx[:, 10 : G + 10, 5 : 5 + WB], op=MAX
                )

                nc.scalar.dma_start(yout[b], acc[:, :, :])

    nc.compile()
    return nc


def _get_nc():
    if "nc" not in _CACHE:
        _CACHE["nc"] = _build()
    return _CACHE["nc"]


def _pack_input(img):
    """[2048, 2048] -> [NSB, 128, XR, WH] with reflect pad + vertical halos.

    Partition p of superband s holds rows [G*g-5, G*g+G+5) and cols
    [(NCHUNK*s + c)*WB - 5, ... + WB + 5) of the original image, where
    c = p // NG, g = p % NG (indices in reflect-padded coordinates).
    """
    xpad = np.pad(img, ((RAD, RAD), (RAD, RAD)), mode="reflect")  # [2058, 2058]
    wv = np.lib.stride_tricks.sliding_window_view(xpad, XR, axis=0)  # [H+11-XR, 2058, XR]
    wv = wv[::G].transpose(0, 2, 1)  # [NG, XR, 2058]
    out = np.empty((NSB, P, XR, WH), dtype=np.float32)
    for s in range(NSB):
        for c in range(NCHUNK):
            j0 = (NCHUNK * s + c) * WB
            out[s, c * NG : (c + 1) * NG] = wv[:, :, j0 : j0 + WH]
    return out


def _unpack_output(yblk):
    """[NSB, 128, G, WB] -> [2048, 2048]."""
    y = np.empty((H, W), dtype=np.float32)
    for s in range(NSB):
        for c in range(NCHUNK):
            j0 = (NCHUNK * s + c) * WB
            blk = yblk[s, c * NG : (c + 1) * NG]  # [NG, G, WB]
            y[:, j0 : j0 + WB] = blk.reshape(H, WB)
    return y


def kernel(x, radius):
    from concourse.bass_utils import run_bass_kernel_spmd

    assert int(radius) == RAD
    x = np.asarray(x, dtype=np.float32)
    B, C = x.shape[0], x.shape[1]
    imgs = x.reshape(B * C, H, W)
    assert imgs.shape[0] == N_CORES

    imgs = np.where(np.isnan(imgs), np.float32(-99.0), imgs)

    nc = _get_nc()
    in_maps = [{"xin": _pack_input(imgs[c])} for c in range(N_CORES)]
    res = run_bass_kernel_spmd(nc, in_maps, core_ids=list(range(N_CORES)), trace=False)
    out = np.empty((N_CORES, H, W), dtype=np.float32)
    for c in range(N_CORES):
        out[c] = _unpack_output(res.results[c]["yout"])
    out = out.reshape(B, C, H, W)
    out = np.where(out == np.float32(-99.0), np.float32(np.nan), out)
    return out.astype(np.float32)



# revision 4
# speedup vs baseline: 2.0613x; 2.0613x over previous
"""CircularMaxPool2d (disk stencil, radius 5, reflect padding) on 8 TRN2 NeuronCores.

Input x: [8, 1, 2048, 2048] f32. Data-parallel: core c processes batch c.

Disk decomposition (radius 5; row widths by |dy|: 11,9,9,9,7,1):

  out[r,c] = max( h11[r,c], max_{|d|<=3} h9[r+d,c], h7[r-4,c], h7[r+4,c],
                  x[r-5,c], x[r+5,c] )

where hk = horizontal k-wide centered running max of x. The horizontal
maxes nest: s1 (2w) -> s2 (4w) -> h7 = s2 over +3 -> h9 = h7 over +-1 ->
h11 = h9 over +-1 (5 ops for all three widths). Vertical: 2-level ladder
t1/t2 over h9 plus direct taps. 13 DVE ops per tile, ~13.7 elem/output.

All compute is fp16 (output tolerance 2e-2, fp16 rounding ~5e-4): DVE runs
16-bit packed tensor_tensor at 2 elem/cycle/lane (2x_1p mode). Only DVE can
run tensor_tensor max on TRN2 (the Pool/GPSIMD engine ISA rejects it), so
the kernel is a single DVE stream with DMA double-buffered underneath.

Layout: each partition owns a (column-chunk, row-group) pair: G=128
consecutive rows x WB=64 columns. The input is packed on the host into a
blocked [superband, 128, G+10, WB+10] fp16 tensor with vertical halo rows
and reflect padding baked in, so every HBM load is fully contiguous and
every vertical shift is a free-dim offset. No on-device halo exchange.
Output is written blocked fp16 and unscrambled on the host.
"""

import sys

sys.path.insert(0, "/opt/trn_rl_repo")

import numpy as np

H = 2048
W = 2048
RAD = 5
P = 128
G = 128  # rows per partition group
NG = H // G  # 16 row groups
NCHUNK = P // NG  # 8 column chunks per superband
WB = 64  # cols per chunk
WH = WB + 2 * RAD  # 74
NSB = W // (WB * NCHUNK)  # 4 superbands
XR = G + 2 * RAD  # 138 rows in x tile
N_CORES = 8

_CACHE = {}


def _build():
    import concourse.bacc as bacc
    import concourse.tile as tile
    import concourse.mybir as mybir

    f16 = mybir.dt.float16
    MAX = mybir.AluOpType.max

    nc = bacc.Bacc("TRN2", target_bir_lowering=False, debug=False, num_devices=N_CORES)
    xin = nc.dram_tensor("xin", [NSB, P, XR, WH], f16, kind="ExternalInput").ap()
    yout = nc.dram_tensor("yout", [NSB, P, G, WB], f16, kind="ExternalOutput").ap()

    with tile.TileContext(nc) as tc:
        with (
            tc.tile_pool(name="xx", bufs=2) as p_xx,
            tc.tile_pool(name="pA", bufs=1) as p_a,
            tc.tile_pool(name="pB", bufs=1) as p_b,
            tc.tile_pool(name="pC", bufs=1) as p_c,
            tc.tile_pool(name="acc", bufs=2) as p_acc,
        ):
            for b in range(NSB):
                # ---- load packed band (halos pre-baked); rows: slot i = row+5
                xx = p_xx.tile([P, XR, WH], f16, tag="xx")
                # s1[i,j] = max over x cols {j-5, j-4} at row i-4 (slot i+1 in xx)
                s1 = p_a.tile([P, G + 8, 73], f16, tag="A")
                if b == 0:
                    # split the cold-start load so the ladder starts after the
                    # first half lands
                    hs = XR // 2  # 69
                    nc.sync.dma_start(xx[:, 0:hs, :], xin[b][:, 0:hs, :])
                    nc.sync.dma_start(xx[:, hs:XR, :], xin[b][:, hs:XR, :])
                    nc.vector.tensor_tensor(
                        s1[:, 0 : hs - 1, :],
                        xx[:, 1:hs, 0:73],
                        xx[:, 1:hs, 1:74],
                        op=MAX,
                    )
                    nc.vector.tensor_tensor(
                        s1[:, hs - 1 : G + 8, :],
                        xx[:, hs : G + 9, 0:73],
                        xx[:, hs : G + 9, 1:74],
                        op=MAX,
                    )
                else:
                    nc.sync.dma_start(xx[:, :, :], xin[b])
                    nc.vector.tensor_tensor(
                        s1[:, :, :],
                        xx[:, 1 : G + 9, 0:73],
                        xx[:, 1 : G + 9, 1:74],
                        op=MAX,
                    )
                # s2[i,j] = 4-wide, x cols j-5..j-2, rows -4..G+3 (slot i = row+4)
                s2 = p_b.tile([P, G + 8, 71], f16, tag="B")
                nc.vector.tensor_tensor(
                    s2[:, :, :], s1[:, :, 0:71], s1[:, :, 2:73], op=MAX
                )
                # h7[i,j] = 7-wide centered at col j-2, rows -4..G+3
                h7 = p_c.tile([P, G + 8, 68], f16, tag="C")
                nc.vector.tensor_tensor(
                    h7[:, :, :], s2[:, :, 0:68], s2[:, :, 3:71], op=MAX
                )
                # h9[k,j] = 9-wide centered at col j-1, rows -3..G+2 (slot k = row+3)
                h9 = p_a.tile([P, G + 6, 66], f16, tag="A")
                nc.vector.tensor_tensor(
                    h9[:, :, :], h7[:, 1 : G + 7, 0:66], h7[:, 1 : G + 7, 2:68], op=MAX
                )
                # h11 (11-wide centered, dy=0) straight into the accumulator
                acc = p_acc.tile([P, G, WB], f16, tag="acc")
                nc.vector.tensor_tensor(
                    acc[:, :, :],
                    h9[:, 3 : G + 3, 0:WB],
                    h9[:, 3 : G + 3, 2 : 2 + WB],
                    op=MAX,
                )
                # t1[m,c] = max(h9[m], h9[m+1]) -> rows m-3..m-2, 9-wide center c
                t1 = p_b.tile([P, G + 5, WB], f16, tag="B")
                nc.vector.tensor_tensor(
                    t1[:, :, :],
                    h9[:, 0 : G + 5, 1 : 1 + WB],
                    h9[:, 1 : G + 6, 1 : 1 + WB],
                    op=MAX,
                )
                # t2[m,c] = max(t1[m], t1[m+2]) -> rows m-3..m
                t2 = p_a.tile([P, G + 3, WB], f16, tag="A")
                nc.vector.tensor_tensor(
                    t2[:, :, :], t1[:, 0 : G + 3, :], t1[:, 2 : G + 5, :], op=MAX
                )
                # acc = max(acc, t2[r], t2[r+3]) -> h9 rows r-3..r+3
                nc.vector.tensor_tensor(acc[:], acc[:], t2[:, 0:G, :], op=MAX)
                nc.vector.tensor_tensor(acc[:], acc[:], t2[:, 3 : G + 3, :], op=MAX)
                # h7 taps at dy = -4, +4 (h7 slot = r-+4 + 4, col center c -> j=c+2)
                nc.vector.tensor_tensor(
                    acc[:], acc[:], h7[:, 0:G, 2 : 2 + WB], op=MAX
                )
                nc.vector.tensor_tensor(
                    acc[:], acc[:], h7[:, 8 : G + 8, 2 : 2 + WB], op=MAX
                )
                # x taps at dy = -5, +5 (xx slot = r-+5 + 5, col slot c+5)
                nc.vector.tensor_tensor(
                    acc[:], acc[:], xx[:, 0:G, 5 : 5 + WB], op=MAX
                )
                nc.vector.tensor_tensor(
                    acc[:], acc[:], xx[:, 10 : G + 10, 5 : 5 + WB], op=MAX
                )

                nc.scalar.dma_start(yout[b], acc[:, :, :])

    nc.compile()
    return nc


def _get_nc():
    if "nc" not in _CACHE:
        _CACHE["nc"] = _build()
    return _CACHE["nc"]


def _pack_input(img):
    """[2048, 2048] f32 -> [NSB, 128, XR, WH] fp16 with reflect pad + halos.

    Partition p of superband s holds rows [G*g-5, G*g+G+5) and cols
    [(NCHUNK*s + c)*WB - 5, ... + WB + 5) of the original image, where
    c = p // NG, g = p % NG (indices in reflect-padded coordinates).
    """
    xpad = np.pad(img, ((RAD, RAD), (RAD, RAD)), mode="reflect")  # [2058, 2058]
    wv = np.lib.stride_tricks.sliding_window_view(xpad, XR, axis=0)
    wv = wv[::G].transpose(0, 2, 1)  # [NG, XR, 2058]
    out = np.empty((NSB, P, XR, WH), dtype=np.float16)
    for s in range(NSB):
        for c in range(NCHUNK):
            j0 = (NCHUNK * s + c) * WB
            out[s, c * NG : (c + 1) * NG] = wv[:, :, j0 : j0 + WH]
    return out


def _unpack_output(yblk):
    """[NSB, 128, G, WB] fp16 -> [2048, 2048] f32."""
    y = np.empty((H, W), dtype=np.float32)
    for s in range(NSB):
        for c in range(NCHUNK):
            j0 = (NCHUNK * s + c) * WB
            blk = yblk[s, c * NG : (c + 1) * NG]  # [NG, G, WB]
            y[:, j0 : j0 + WB] = blk.reshape(H, WB).astype(np.float32)
    return y


def kernel(x, radius):
    from concourse.bass_utils import run_bass_kernel_spmd

    assert int(radius) == RAD
    x = np.asarray(x, dtype=np.float32)
    B, C = x.shape[0], x.shape[1]
    imgs = x.reshape(B * C, H, W)
    assert imgs.shape[0] == N_CORES

    imgs = np.where(np.isnan(imgs), np.float32(-99.0), imgs)

    nc = _get_nc()
    in_maps = [{"xin": _pack_input(imgs[c])} for c in range(N_CORES)]
    res = run_bass_kernel_spmd(nc, in_maps, core_ids=list(range(N_CORES)), trace=False)
    out = np.empty((N_CORES, H, W), dtype=np.float32)
    for c in range(N_CORES):
        out[c] = _unpack_output(res.results[c]["yout"])
    out = out.reshape(B, C, H, W)
    out = np.where(out == np.float32(-99.0), np.float32(np.nan), out)
    return out.astype(np.float32)


# revision 6
# speedup vs baseline: 2.1056x; 1.0215x over previous
"""CircularMaxPool2d (disk stencil, radius 5, reflect padding) on 8 TRN2 NeuronCores.

Input x: [8, 1, 2048, 2048] f32. Data-parallel: core c processes batch c.

Disk decomposition (radius 5; row widths by |dy|: 11,9,9,9,7,1):

  out[r,c] = max( h11[r,c], max_{|d|<=3} h9[r+d,c], h7[r-4,c], h7[r+4,c],
                  x[r-5,c], x[r+5,c] )

where hk = horizontal k-wide centered running max of x. The horizontal
maxes nest: s1 (2w) -> s2 (4w) -> h7 = s2 over +3 -> h9 = h7 over +-1 ->
h11 = h9 over +-1 (5 ops for all three widths). Vertical: 2-level ladder
t1/t2 over h9 plus direct taps. 13 DVE ops per tile, ~13.7 elem/output.

All compute is fp16 (output tolerance 2e-2, fp16 rounding ~5e-4): DVE runs
16-bit packed tensor_tensor at 2 elem/cycle/lane (2x_1p mode). Only DVE can
run tensor_tensor max on TRN2 (the Pool/GPSIMD engine ISA rejects it), so
the kernel is a single DVE stream with DMA double-buffered underneath.

Layout: each partition owns a (column-chunk, row-group) pair: G=128
consecutive rows x WB=64 columns. The input is packed on the host into a
blocked [superband, 128, G+10, WB+10] fp16 tensor with vertical halo rows
and reflect padding baked in, so every HBM load is fully contiguous and
every vertical shift is a free-dim offset. No on-device halo exchange.
Output is written blocked fp16 and unscrambled on the host.
"""

import sys

sys.path.insert(0, "/opt/trn_rl_repo")

import numpy as np

H = 2048
W = 2048
RAD = 5
P = 128
G = 128  # rows per partition group
NG = H // G  # 16 row groups
NCHUNK = P // NG  # 8 column chunks per superband
WB = 64  # cols per chunk
WH = WB + 2 * RAD  # 74
NSB = W // (WB * NCHUNK)  # 4 superbands
XR = G + 2 * RAD  # 138 rows in x tile
N_CORES = 8

_CACHE = {}


def _build():
    import concourse.bacc as bacc
    import concourse.tile as tile
    import concourse.mybir as mybir

    f16 = mybir.dt.float16
    MAX = mybir.AluOpType.max

    nc = bacc.Bacc("TRN2", target_bir_lowering=False, debug=False, num_devices=N_CORES)
    xin = nc.dram_tensor("xin", [NSB, P, XR, WH], f16, kind="ExternalInput").ap()
    yout = nc.dram_tensor("yout", [NSB, P, G, WB], f16, kind="ExternalOutput").ap()

    with tile.TileContext(nc) as tc:
        with (
            tc.tile_pool(name="xx", bufs=2) as p_xx,
            tc.tile_pool(name="pA", bufs=1) as p_a,
            tc.tile_pool(name="pB", bufs=1) as p_b,
            tc.tile_pool(name="pC", bufs=1) as p_c,
            tc.tile_pool(name="acc", bufs=2) as p_acc,
        ):
            for b in range(NSB):
                # ---- load packed band (halos pre-baked); rows: slot i = row+5
                xx = p_xx.tile([P, XR, WH], f16, tag="xx")
                # s1[i,j] = max over x cols {j-5, j-4} at row i-4 (slot i+1 in xx)
                s1 = p_a.tile([P, G + 8, 73], f16, tag="A")
                if b == 0:
                    # split the cold-start load 4 ways so the ladder starts
                    # after the first quarter lands
                    cuts = [0, 35, 69, 104, XR]
                    for k in range(4):
                        nc.sync.dma_start(
                            xx[:, cuts[k] : cuts[k + 1], :],
                            xin[b][:, cuts[k] : cuts[k + 1], :],
                        )
                    # s1 chunk k needs xx rows [cuts[k-1]+1, cuts[k]+1) ... i.e.
                    # chunk k of s1 rows [cuts[k]-1, cuts[k+1]-1) reads xx rows
                    # [cuts[k], cuts[k+1]) plus one row from the next chunk; use
                    # rows [max(cuts[k]-1,0), min(cuts[k+1]-1, G+8)) so chunk k
                    # only reads xx rows < cuts[k+1]+... keep it simple: chunk k
                    # covers s1 rows [lo, hi) with lo=cuts[k]-1 (clamped), which
                    # reads xx rows [lo+1, hi+1) <= cuts[k+1].
                    for k in range(4):
                        lo = max(cuts[k] - 1, 0)
                        hi = min(cuts[k + 1] - 1, G + 8)
                        nc.vector.tensor_tensor(
                            s1[:, lo:hi, :],
                            xx[:, lo + 1 : hi + 1, 0:73],
                            xx[:, lo + 1 : hi + 1, 1:74],
                            op=MAX,
                        )
                else:
                    nc.sync.dma_start(xx[:, :, :], xin[b])
                    nc.vector.tensor_tensor(
                        s1[:, :, :],
                        xx[:, 1 : G + 9, 0:73],
                        xx[:, 1 : G + 9, 1:74],
                        op=MAX,
                    )
                # s2[i,j] = 4-wide, x cols j-5..j-2, rows -4..G+3 (slot i = row+4)
                s2 = p_b.tile([P, G + 8, 71], f16, tag="B")
                nc.vector.tensor_tensor(
                    s2[:, :, :], s1[:, :, 0:71], s1[:, :, 2:73], op=MAX
                )
                # h7[i,j] = 7-wide centered at col j-2, rows -4..G+3
                h7 = p_c.tile([P, G + 8, 68], f16, tag="C")
                nc.vector.tensor_tensor(
                    h7[:, :, :], s2[:, :, 0:68], s2[:, :, 3:71], op=MAX
                )
                # h9[k,j] = 9-wide centered at col j-1, rows -3..G+2 (slot k = row+3)
                h9 = p_a.tile([P, G + 6, 66], f16, tag="A")
                nc.vector.tensor_tensor(
                    h9[:, :, :], h7[:, 1 : G + 7, 0:66], h7[:, 1 : G + 7, 2:68], op=MAX
                )
                # h11 (11-wide centered, dy=0) straight into the accumulator
                acc = p_acc.tile([P, G, WB], f16, tag="acc")
                nc.vector.tensor_tensor(
                    acc[:, :, :],
                    h9[:, 3 : G + 3, 0:WB],
                    h9[:, 3 : G + 3, 2 : 2 + WB],
                    op=MAX,
                )
                # t1[m,c] = max(h9[m], h9[m+1]) -> rows m-3..m-2, 9-wide center c
                t1 = p_b.tile([P, G + 5, WB], f16, tag="B")
                nc.vector.tensor_tensor(
                    t1[:, :, :],
                    h9[:, 0 : G + 5, 1 : 1 + WB],
                    h9[:, 1 : G + 6, 1 : 1 + WB],
                    op=MAX,
                )
                # t2[m,c] = max(t1[m], t1[m+2]) -> rows m-3..m
                t2 = p_a.tile([P, G + 3, WB], f16, tag="A")
                nc.vector.tensor_tensor(
                    t2[:, :, :], t1[:, 0 : G + 3, :], t1[:, 2 : G + 5, :], op=MAX
                )
                # acc = max(acc, t2[r], t2[r+3])       -> h9 rows r-3..r+3
                #       max(acc, h7[r-4], h7[r+4])     (h7 slot = r-+4+4, col j=c+2)
                #       max(acc, x[r-5], x[r+5])       (xx slot = r-+5+5, col c+5)
                # On the last superband, run the taps in two row-halves and
                # store the first half early to hide the output DMA tail.
                halves = [(0, G)] if b < NSB - 1 else [(0, G // 2), (G // 2, G)]
                for h0, h1 in halves:
                    a = acc[:, h0:h1, :]
                    nc.vector.tensor_tensor(a, a, t2[:, h0:h1, :], op=MAX)
                    nc.vector.tensor_tensor(a, a, t2[:, 3 + h0 : 3 + h1, :], op=MAX)
                    nc.vector.tensor_tensor(a, a, h7[:, h0:h1, 2 : 2 + WB], op=MAX)
                    nc.vector.tensor_tensor(
                        a, a, h7[:, 8 + h0 : 8 + h1, 2 : 2 + WB], op=MAX
                    )
                    nc.vector.tensor_tensor(a, a, xx[:, h0:h1, 5 : 5 + WB], op=MAX)
                    nc.vector.tensor_tensor(
                        a, a, xx[:, 10 + h0 : 10 + h1, 5 : 5 + WB], op=MAX
                    )
                    nc.scalar.dma_start(
                        yout[b][:, h0:h1, :], acc[:, h0:h1, :]
                    )

    nc.compile()
    return nc


def _get_nc():
    if "nc" not in _CACHE:
        _CACHE["nc"] = _build()
    return _CACHE["nc"]


def _pack_input(img):
    """[2048, 2048] f32 -> [NSB, 128, XR, WH] fp16 with reflect pad + halos.

    Partition p of superband s holds rows [G*g-5, G*g+G+5) and cols
    [(NCHUNK*s + c)*WB - 5, ... + WB + 5) of the original image, where
    c = p // NG, g = p % NG (indices in reflect-padded coordinates).
    """
    xpad = np.pad(img, ((RAD, RAD), (RAD, RAD)), mode="reflect")  # [2058, 2058]
    wv = np.lib.stride_tricks.sliding_window_view(xpad, XR, axis=0)
    wv = wv[::G].transpose(0, 2, 1)  # [NG, XR, 2058]
    out = np.empty((NSB, P, XR, WH), dtype=np.float16)
    for s in range(NSB):
        for c in range(NCHUNK):
            j0 = (NCHUNK * s + c) * WB
            out[s, c * NG : (c + 1) * NG] = wv[:, :, j0 : j0 + WH]
    return out


def _unpack_output(yblk):
    """[NSB, 128, G, WB] fp16 -> [2048, 2048] f32."""
    y = np.empty((H, W), dtype=np.float32)
    for s in range(NSB):
        for c in range(NCHUNK):
            j0 = (NCHUNK * s + c) * WB
            blk = yblk[s, c * NG : (c + 1) * NG]  # [NG, G, WB]
            y[:, j0 : j0 + WB] = blk.reshape(H, WB).astype(np.float32)
    return y


def kernel(x, radius):
    from concourse.bass_utils import run_bass_kernel_spmd

    assert int(radius) == RAD
    x = np.asarray(x, dtype=np.float32)
    B, C = x.shape[0], x.shape[1]
    imgs = x.reshape(B * C, H, W)
    assert imgs.shape[0] == N_CORES

    imgs = np.where(np.isnan(imgs), np.float32(-99.0), imgs)

    nc = _get_nc()
    in_maps = [{"xin": _pack_input(imgs[c])} for c in range(N_CORES)]
    res = run_bass_kernel_spmd(nc, in_maps, core_ids=list(range(N_CORES)), trace=False)
    out = np.empty((N_CORES, H, W), dtype=np.float32)
    for c in range(N_CORES):
        out[c] = _unpack_output(res.results[c]["yout"])
    out = out.reshape(B, C, H, W)
    out = np.where(out == np.float32(-99.0), np.float32(np.nan), out)
    return out.astype(np.float32)
